# revision 46
# baseline (speedup 1.0000x reference)
"""Trainium2 Bass kernel for 16-head MHA (B=2, T=2048, C=1024).

Sharding: 8 cores = 2 batches x 4 head-groups (4 heads each).
Each core computes, for its batch b and head group g:
  partialT[c, t] = sum_{h in g} wo[:, h].T @ (softmax(qk^T) @ v_h)^T
in fully transposed space (no on-device transposes needed):
  - host passes xT = x[b].T, weight slices pre-tiled to [128, ...] with
    4KB DMA lines, and a consts block (mask bias / Schraudolph bias /
    q,k biases) as one [128, 36] tensor
  - qT/kT computed as [d, t]; v as [t, d] (+ ones column per head for the
    softmax denominator); scores computed directly as [tk, tq]
  - denominator emerges as row 64 of the attn@v_ext matmul output;
    normalization folded in as a K=1 "replicate" matmul + multiply
  - final projection consumes the [d, t] head outputs as stationary weights
Host adds the 4 partial sums per batch, the wo bias, and the wv_b @ wo.T
constant row (v-bias contribution commutes through softmax normalization).

Schedule: heads are processed in PAIRS (SBUF partition base 0 / 64).  The
two K=64 score matmuls of a pair land on disjoint PE row-groups
(tile_position (0,0) / (64,0)) and execute concurrently, halving score
wall time.  A pass covers one 512-query chunk for both heads of a pair;
the exp runs as one [128, 1024] ACT instruction per key tile (~1.1us),
the pacing engine.  In the ACT-bound passes a few key tiles' exp runs on
the Vector engine instead, via the Schraudolph bit trick
(int32(A*(s+bias+B/A)) reinterpreted as float32, end-to-end rel err
~3e-3): one tensor_scalar into an int32 tile consumed through a
bitcast AP.  Projection / v / norm work fills the remaining PE slack per
round; softmax reciprocals run at [128, 4] after a scatter DMA.  Output
partials are stored fp16; po stores ride the gpsimd queue so the norm
DMAs on sync are never queued behind them.
"""

import sys

sys.path.insert(0, "/opt/trn_rl_repo")

import numpy as np

# ---- problem constants (hardcoded per harness contract) ----
B = 2
T = 2048
C = 1024
NUM_HEADS = 16
G = 4                 # head groups (tensor-parallel dimension)
HPG = NUM_HEADS // G  # 4 heads per core
DH = C // NUM_HEADS   # 64
DC = HPG * DH         # 256 dims per core
VE = HPG * (DH + 1)   # 260: per head 64 v-dims + 1 ones column
N_CORES = B * G       # 8
PAD_ID = 0.0

CH = 512              # tq chunk (one PSUM bank of fp32)
NCH = T // CH         # 4
NT = T // 128         # 16 token tiles
KT = C // 128         # 8 contraction tiles for projections
DM = DC // 128        # 2 m-tiles for q/k (== head pairs)

PASS_ORDER = [(0, 0), (0, 1), (1, 0), (1, 1), (0, 2), (0, 3), (1, 2), (1, 3)]

# Schraudolph exp constants (exp(x) ~ bitcast(int32(A*x + B)))
EXP_A = float((1 << 23) / np.log(2.0))
EXP_B = float((127 << 23) - 400000)
# DVE-offloaded key tiles per pass (pass idx -> rounds); heavier in the
# filler-light passes 5/6 where ACT would otherwise pace the rounds
DVE_EXP = {2: (3, 6, 10, 14), 3: (7, 13), 4: (7, 13),
           5: (2, 5, 7, 9, 11, 13), 6: (2, 5, 7, 9, 11, 13), 7: (7, 13)}


def build_nc(debug=False):
    import concourse.tile as tile
    from concourse import bacc, mybir

    f32 = mybir.dt.float32
    f32r = mybir.dt.float32r
    f16 = mybir.dt.float16
    i32 = mybir.dt.int32
    Exp = mybir.ActivationFunctionType.Exp
    add = mybir.AluOpType.add
    mult = mybir.AluOpType.mult

    nc = bacc.Bacc(
        "TRN2", target_bir_lowering=False, debug=debug, num_devices=N_CORES
    )

    xT_d = nc.dram_tensor("xT", [C, T], f16, kind="ExternalInput")
    wqT_d = nc.dram_tensor("wqT", [128, KT * DC], f16, kind="ExternalInput")
    wkT_d = nc.dram_tensor("wkT", [128, KT * DC], f16, kind="ExternalInput")
    wvT_d = nc.dram_tensor("wvT", [128, KT * VE], f16, kind="ExternalInput")
    woT_d = nc.dram_tensor("woT", [DC, C], f16, kind="ExternalInput")
    ones_d = nc.dram_tensor("ones", [128, DH], f32r, kind="ExternalInput")
    consts_d = nc.dram_tensor("consts", [128, 36], f32, kind="ExternalInput")
    outT_d = nc.dram_tensor("outT", [C, T], f16, kind="ExternalOutput")

    from contextlib import ExitStack

    with tile.TileContext(nc) as tc, ExitStack() as stack:
        persist = stack.enter_context(tc.tile_pool(name="persist", bufs=1))
        psum = stack.enter_context(tc.tile_pool(name="psum", bufs=1, space="PSUM"))
        xpool = stack.enter_context(tc.tile_pool(name="xpool", bufs=1))
        atpool = stack.enter_context(tc.tile_pool(name="atpool", bufs=1))
        work = stack.enter_context(tc.tile_pool(name="work", bufs=1))

        # PSUM bank plan (8 banks of [128, 2KB]):
        #   sc  : [128, 1024] f32 x bufs=2  -> 4 banks (score pair tiles)
        #   un0 : [128, 512] f32 x bufs=1   -> 1 bank  (attn@v head 0)
        #   un1 : [128, 512] f32 x bufs=1   -> 1 bank  (attn@v head 1)
        #   pp  : [128, 512] f32 x bufs=2   -> 2 banks (q/k/v/wo proj + rb)

        # ---------- loads (sync queue: consts + packed weights) ----------
        consts = persist.tile([128, 36], f32, name="consts", tag="consts")
        nc.sync.dma_start(consts[:, :], consts_d.ap()[:, :])
        mb = consts[:, 0:NT]                  # ACT exp bias (mask - 2)
        mbB = consts[:, NT:2 * NT]            # DVE exp bias (mask - 2 + B/A)
        bqt = [consts[:, 32 + m:33 + m] for m in range(DM)]
        bkt = [consts[:, 34 + m:35 + m] for m in range(DM)]

        # x half-tile helper (A = cols 0:1024, B = cols 1024:2048); sync is
        # the fastest queue (~190GB/s) so the ramp-gating halves ride it
        # interleaved with the weights; scalar measures only ~55GB/s
        xs = [None] * KT
        for k in range(KT):
            xs[k] = xpool.tile([128, T], f16, name=f"x{k}", tag=f"x{k}")
        HT = T // 2

        def xdma(eng, k, half):
            eng.dma_start(
                xs[k][:, half * HT:(half + 1) * HT],
                xT_d.ap()[k * 128:(k + 1) * 128, half * HT:(half + 1) * HT],
            )

        wk_t = persist.tile([128, KT * DC], f16, name="wk_t", tag="wk_t")
        nc.sync.dma_start(wk_t[:, :], wkT_d.ap()[:, :])
        xdma(nc.scalar, 0, 0)
        xdma(nc.scalar, 2, 0)
        for k in (3, 5, 7):
            xdma(nc.gpsimd, k, 0)
        xdma(nc.sync, 1, 0)
        wq_t = persist.tile([128, KT * DC], f16, name="wq_t", tag="wq_t")
        nc.sync.dma_start(wq_t[:, :], wqT_d.ap()[:, :])
        xdma(nc.sync, 4, 0)
        xdma(nc.sync, 6, 0)
        wv_t = persist.tile([128, KT * VE], f16, name="wv_t", tag="wv_t")
        nc.sync.dma_start(wv_t[:, :], wvT_d.ap()[:, :])
        xdma(nc.scalar, 0, 1)
        xdma(nc.scalar, 6, 1)
        xdma(nc.sync, 2, 1)
        for k in (1, 3, 5, 7):
            xdma(nc.gpsimd, k, 1)
        xdma(nc.sync, 4, 1)
        ones64 = persist.tile([1, DH], f32r, name="ones64", tag="ones64")
        nc.sync.dma_start(ones64[:, :], ones_d.ap()[0:1, :])
        wo_sb = []
        for k2 in range(DM):
            wok = persist.tile([128, C], f16, name=f"wo{k2}", tag=f"wo{k2}")
            nc.sync.dma_start(wok[:, :], woT_d.ap()[k2 * 128:(k2 + 1) * 128, :])
            wo_sb.append(wok)

        # warm the ACT exp table set (~2.7us) behind the x descriptors
        expwarm = work.tile([1, 1], f32, name="expwarm", tag="expwarm", bufs=1)
        nc.vector.memset(expwarm[:, :], 0.0)
        nc.scalar.activation(expwarm[:, :], expwarm[:, :], Exp)

        # ---------- projection / v building blocks ----------
        qT = [
            persist.tile([128, T], f16, name=f"qT{m}", tag=f"qT{m}")
            for m in range(DM)
        ]
        kT = [
            persist.tile([128, T], f16, name=f"kT{m}", tag=f"kT{m}")
            for m in range(DM)
        ]
        headsT = [
            persist.tile([128, T], f16, name=f"headsT{m}", tag=f"hT{m}")
            for m in range(DM)
        ]

        def pp_tile(name):
            return psum.tile([128, CH], f32, name=name, tag="pp", bufs=2)

        # chain contraction order matched to x-tile DMA arrival
        K_ORDER = (1, 3, 5, 7, 0, 4, 6, 2)

        def proj_chain(dst, w_t, bias, m, ch):
            # one (dst, m, ch) chain: 4 two-matmul sub-units, bias evict
            state = {}
            units = []
            for step in range(4):
                def u(dst=dst, w_t=w_t, bias=bias, ch=ch,
                      state=state, m=m, step=step):
                    if step == 0:
                        state["ps"] = pp_tile(f"ps{dst[0].name}{m}{ch}")
                    ps = state["ps"]
                    for k in K_ORDER[2 * step:2 * step + 2]:
                        nc.tensor.matmul(
                            ps[:, :],
                            w_t[:, k * DC + m * 128:k * DC + (m + 1) * 128],
                            xs[k][:, ch * CH:(ch + 1) * CH],
                            start=(k == K_ORDER[0]),
                            stop=(k == K_ORDER[-1]),
                        )
                    if step == 3:
                        nc.vector.tensor_scalar_add(
                            dst[m][:, ch * CH:(ch + 1) * CH],
                            ps[:, :],
                            bias[m],
                        )
                units.append(u)
            return units

        def q_units(m, ch):
            return proj_chain(qT, wq_t, bqt, m, ch)

        def k_units(m, ch):
            return proj_chain(kT, wk_t, bkt, m, ch)

        v_sb = [None] * NT

        def v_subunits(tkt):
            state = {}
            units = []
            for step in range(2):
                def u(tkt=tkt, state=state, step=step):
                    if step == 0:
                        state["ps"] = pp_tile(f"psv{tkt}")
                    psv = state["ps"]
                    for k in K_ORDER[4 * step:4 * step + 4]:
                        nc.tensor.matmul(
                            psv[:, 0:VE],
                            xs[k][:, tkt * 128:(tkt + 1) * 128],
                            wv_t[:, k * VE:(k + 1) * VE],
                            start=(k == K_ORDER[0]),
                            stop=(k == K_ORDER[-1]),
                        )
                    if step == 1:
                        vt = persist.tile(
                            [128, VE], f16, name=f"v{tkt}", tag=f"v{tkt}"
                        )
                        nc.vector.tensor_copy(vt[:, :], psv[:, 0:VE])
                        ones_cols = vt.rearrange(
                            "p (h e) -> p h e", e=DH + 1
                        )[:, :, DH]
                        nc.vector.memset(ones_cols, 1.0)
                        v_sb[tkt] = vt
                units.append(u)
            return units

        def proj_units(mc, ch):
            # wo-projection for output rows [mc*128, ...) of chunk ch,
            # split into two ~512-cycle halves; po stores on gpsimd queue
            state = {}

            def u0():
                state["pp"] = pp_tile(f"pp{mc}{ch}")
                nc.tensor.matmul(
                    state["pp"][:, :],
                    wo_sb[0][:, mc * 128:(mc + 1) * 128],
                    headsT[0][:, ch * CH:(ch + 1) * CH],
                    start=True, stop=False,
                )

            def u1():
                pp = state["pp"]
                nc.tensor.matmul(
                    pp[:, :],
                    wo_sb[1][:, mc * 128:(mc + 1) * 128],
                    headsT[1][:, ch * CH:(ch + 1) * CH],
                    start=False, stop=True,
                )
                po = work.tile(
                    [128, CH], f16, name=f"po{mc}{ch}", tag="po", bufs=4
                )
                nc.scalar.copy(po[:, :], pp[:, :])
                nc.sync.dma_start(
                    outT_d.ap()[
                        mc * 128:(mc + 1) * 128, ch * CH:(ch + 1) * CH
                    ],
                    po[:, :],
                )
            return u0, u1

        # ---------- norm (softmax denominator) ----------
        pending_norm = []
        tail_rb = {"on": False, "n": 0}

        def make_norm_part2(p, ch, hh, unev, rr):
            base = hh * 64

            def emit():
                if tail_rb["on"]:
                    # tail: pp bufs are held by proj-c3 first halves; the
                    # un banks are free after the final evicts
                    rb = psum.tile(
                        [128, CH], f32, name=f"rbt{p}{ch}{hh}",
                        tag=f"un{tail_rb['n'] % 2}", bufs=1,
                    )
                    tail_rb["n"] += 1
                else:
                    rb = pp_tile(f"rb{p}{ch}{hh}")
                nc.tensor.matmul(
                    rb[0:DH, :], ones64[:, :], rr[:, :], start=True, stop=True
                )
                if base == 0:
                    nc.vector.tensor_mul(
                        headsT[p][0:DH, ch * CH:(ch + 1) * CH],
                        unev[0:DH, :],
                        rb[0:DH, :],
                    )
                else:
                    scr = work.tile(
                        [DH, CH], f16, name=f"scr{p}{ch}{hh}", tag="scr",
                        bufs=4,
                    )
                    nc.vector.tensor_mul(scr[:, :], unev[0:DH, :], rb[0:DH, :])
                    sq = nc.sync if tail_rb["on"] else nc.gpsimd
                    sq.dma_start(
                        headsT[p][base:base + 64, ch * CH:(ch + 1) * CH],
                        scr[:, :],
                    )
            return emit

        def norm_part1(p, ch, hh, un):
            # evict un, spread denominator row across 128 partitions,
            # reciprocal there, gather back to [1, 512] for the rb matmul
            unev = work.tile(
                [DH + 1, CH], f32, name=f"unev{p}{ch}{hh}", tag="unev", bufs=6
            )
            nc.vector.tensor_copy(unev[:, :], un[0:DH + 1, :])
            dq = nc.sync if tail_rb["on"] else nc.gpsimd
            drp = work.tile(
                [128, CH // 128], f32, name=f"drp{p}{ch}{hh}", tag="drp",
                bufs=6,
            )
            dq.dma_start(drp[:, :], unev[DH:DH + 1, :])
            rrp = work.tile(
                [128, CH // 128], f32r, name=f"rrp{p}{ch}{hh}", tag="rrp",
                bufs=6,
            )
            with nc.allow_low_precision(reason="fp32r matmul operand"):
                nc.vector.reciprocal(rrp[:, :], drp[:, :])
            rr = work.tile(
                [1, CH], f32r, name=f"rr{p}{ch}{hh}", tag="rr", bufs=6
            )
            dq.dma_start(rr[:, :], rrp[:, :])
            pending_norm.append(make_norm_part2(p, ch, hh, unev, rr))

        # ---------- minimal ramp: just enough for sc(0) of pass 0 ----------
        for u in k_units(0, 0):
            u()
        for u in q_units(0, 0):
            u()

        # ---------- filler schedule: pass idx -> {round: [closures]} ----------
        filler = {pi: {} for pi in range(len(PASS_ORDER))}

        def sched(pi, t, *fns):
            filler[pi].setdefault(t, []).extend(fns)

        vu = {t: v_subunits(t) for t in range(NT)}
        # pass 0: v(0..15) [deadline: round t], kT[0] c1/c2/c3
        # [deadline: round 4c-2], qT[0] c1 [deadline: end of pass]
        k01, k02, k03 = k_units(0, 1), k_units(0, 2), k_units(0, 3)
        sched(0, 0, *vu[0], *vu[1])
        sched(0, 1, *vu[2], *k01[0:2])
        sched(0, 2, *vu[3], *k01[2:4])
        sched(0, 3, *vu[4])
        sched(0, 4, *vu[5], *k02[0:2])
        sched(0, 5, *vu[6], *k02[2:4])
        sched(0, 6, *vu[7])
        sched(0, 7, *vu[8])
        sched(0, 8, *vu[9], *k03[0:2])
        sched(0, 9, *vu[10], *k03[2:4])
        sched(0, 10, *vu[11])
        sched(0, 11, *vu[12])
        q01 = q_units(0, 1)
        sched(0, 12, *vu[13], q01[0])
        sched(0, 13, *vu[14], q01[1])
        sched(0, 14, *vu[15], q01[2])
        sched(0, 15, q01[3])

        # pass 1: kT[1] c0/c1 (1 unit/round) + qT[1] c0
        for c in range(2):
            ku = k_units(1, c)
            for j in range(4):
                sched(1, 4 * c + j, ku[j])
        q10 = q_units(1, 0)
        for j in range(4):
            sched(1, 8 + 2 * j, q10[j])
        # pass 2: kT[1] c2/c3 (deadline: chunk c read at sc(4c), emitted
        # round 4c-1), then qT[1] c1, qT[0] c2
        k12, k13 = k_units(1, 2), k_units(1, 3)
        for j in range(4):
            sched(2, j, k12[j])
            sched(2, 4 + j, k13[j])
        for j, u in enumerate(q_units(1, 1)):
            sched(2, 8 + j, u)
        # passes 3/4/7: wo-projection for chunks 0/1/2 (halves pipelined)
        for pi, ch in ((3, 0), (4, 1), (7, 2)):
            prev = None
            for mc in range(8):
                u0, u1 = proj_units(mc, ch)
                if prev is None:
                    sched(pi, 4, u0)
                else:
                    sched(pi, 4 + mc, prev, u0)
                prev = u1
            sched(pi, 12, prev)
        for j, u in enumerate(q_units(0, 2)):
            sched(3, 12 + j, u)
        for j, u in enumerate(q_units(0, 3)):
            sched(4, 12 + j, u)
        for j, u in enumerate(q_units(1, 3)):
            sched(5, 2 + 2 * j, u)
        for j, u in enumerate(q_units(1, 2)):
            sched(5, 10 + j, u)

        # norm part2 pops: pass i's norms pop early in pass i+1 (pass 0's
        # and 1's in pass 2).  In proj passes (3/4/7) both pops must land
        # before round 4, where the first proj unit reads headsT.
        pops = {2: (1, 5, 9, 13), 3: (1, 3), 4: (1, 3), 5: (1, 5),
                6: (1, 5), 7: (1, 3)}

        # ---------- attention passes ----------
        tail_units = [proj_units(mc, 3) for mc in range(8)]
        for pi, (p, ch) in enumerate(PASS_ORDER):
            un = [
                psum.tile([128, CH], f32, name=f"un{p}{ch}{hh}",
                          tag=f"un{hh}", bufs=1)
                for hh in range(2)
            ]
            dve_rounds = DVE_EXP.get(pi, ())

            def emit_sc(t, p=p, ch=ch):
                sc = psum.tile(
                    [128, 2 * CH], f32, name=f"sc{p}{ch}t{t}", tag="sc",
                    bufs=2,
                )
                for hh in range(2):
                    base = hh * 64
                    nc.tensor.matmul(
                        sc[:, hh * CH:(hh + 1) * CH],
                        kT[p][base:base + 64, t * 128:(t + 1) * 128],
                        qT[p][base:base + 64, ch * CH:(ch + 1) * CH],
                        start=True,
                        stop=True,
                    )
                return sc

            sc_cur = emit_sc(0)
            for t in range(NT):
                if t in dve_rounds:
                    # Schraudolph exp on DVE: int32((sc + mbB)*A), read back
                    # through a f32 bitcast and narrowed to f16 for the
                    # attn@v matmul (PE rejects f16-stationary/f32-moving)
                    ati = atpool.tile(
                        [128, 2 * CH], i32, name=f"ati{p}{ch}t{t}", tag="ati",
                        bufs=2,
                    )
                    nc.vector.tensor_scalar(
                        out=ati[:, :], in0=sc_cur[:, :],
                        scalar1=mbB[:, t:t + 1], scalar2=EXP_A,
                        op0=add, op1=mult,
                    )
                    at = atpool.tile(
                        [128, 2 * CH], f16, name=f"at{p}{ch}t{t}", tag="at",
                        bufs=6,
                    )
                    nc.vector.tensor_copy(at[:, :], ati[:, :].bitcast(f32))
                    at_slices = [at[:, hh * CH:(hh + 1) * CH] for hh in range(2)]
                else:
                    at = atpool.tile(
                        [128, 2 * CH], f16, name=f"at{p}{ch}t{t}", tag="at",
                        bufs=6,
                    )
                    nc.scalar.activation(
                        at[:, :], sc_cur[:, :], Exp, bias=mb[:, t:t + 1]
                    )
                    at_slices = [at[:, hh * CH:(hh + 1) * CH] for hh in range(2)]
                if t + 1 < NT:
                    sc_cur = emit_sc(t + 1)
                for fn in filler[pi].get(t, ()):
                    fn()
                if t in pops.get(pi, ()) and pending_norm:
                    pending_norm.pop(0)()
                for hh in range(2):
                    h = 2 * p + hh
                    nc.tensor.matmul(
                        un[hh][0:DH + 1, :],
                        v_sb[t][:, h * (DH + 1):(h + 1) * (DH + 1)],
                        at_slices[hh],
                        start=(t == 0),
                        stop=(t == NT - 1),
                    )
                    if t == NT - 1 and pi < 7:
                        norm_part1(p, ch, hh, un[hh])

            if pi == 7:
                # last pass: proj-c3 first halves go out BEFORE the norm
                # DMAs (queue-counting semaphores would otherwise gate the
                # matmuls on them), then the final norms on the idle sync
                # queue
                tail_units[0][0]()
                tail_units[1][0]()
                tail_rb["on"] = True
                for hh in range(2):
                    norm_part1(p, ch, hh, un[hh])

        # ---------- tail: wo-proj chunk 3 ----------
        while pending_norm:
            pending_norm.pop(0)()
        for mc in range(8):
            tail_units[mc][1]()
            if mc + 2 < 8:
                tail_units[mc + 2][0]()

    nc.compile()
    return nc


def make_in_maps(x, wq_w, wq_b, wk_w, wk_b, wv_w, wv_b, wo_w, wo_b):
    scale = DH ** -0.5

    def pack(wT, ve):  # [C, ve*KT-ish] -> [128, KT*ve] tiled rows
        return np.ascontiguousarray(
            wT.reshape(KT, 128, ve).transpose(1, 0, 2).reshape(128, KT * ve)
        )

    in_maps = []
    for c in range(N_CORES):
        b, g = divmod(c, G)
        sl = slice(g * DC, (g + 1) * DC)
        wvT_ext = np.zeros((C, VE), np.float32)
        for hl in range(HPG):
            rows = slice(g * DC + hl * DH, g * DC + (hl + 1) * DH)
            wvT_ext[:, hl * (DH + 1):hl * (DH + 1) + DH] = wv_w[rows, :].T
        # consts [128, 36]: mask bias, Schraudolph bias, bq, bk
        col0 = np.asarray(x[b][:, 0], np.float32).reshape(NT, 128).T
        mbias = np.where(col0 == PAD_ID, -1e30, 0.0).astype(np.float32) - 2.0
        consts = np.zeros((128, 36), np.float32)
        consts[:, 0:NT] = mbias
        consts[:, NT:2 * NT] = mbias + np.float32(EXP_B / EXP_A)
        for m in range(DM):
            consts[:, 32 + m] = (wq_b[sl] * scale)[m * 128:(m + 1) * 128]
            consts[:, 34 + m] = wk_b[sl][m * 128:(m + 1) * 128]
        in_maps.append({
            "xT": np.ascontiguousarray(x[b].T).astype(np.float16),
            "wqT": pack((wq_w[sl] * scale).T, DC).astype(np.float16),
            "wkT": pack(wk_w[sl].T, DC).astype(np.float16),
            "wvT": pack(wvT_ext, VE).astype(np.float16),
            "woT": np.ascontiguousarray(wo_w[:, sl].T).astype(np.float16),
            "ones": np.ones((128, DH), np.float32),
            "consts": consts,
        })
    return in_maps


def assemble_output(results, wv_b, wo_w, wo_b):
    const_row = wv_b @ wo_w.T + wo_b  # [C]
    out = np.zeros((B, T, C), np.float32)
    for c in range(N_CORES):
        b = c // G
        out[b] += results[c]["outT"].astype(np.float32).T
    out += const_row[None, None, :]
    return out.astype(np.float32)


_nc_cache = {}


def kernel(**inputs):
    from concourse.bass_utils import run_bass_kernel_spmd

    if "nc" not in _nc_cache:
        _nc_cache["nc"] = build_nc(debug=False)
    nc = _nc_cache["nc"]

    in_maps = make_in_maps(**inputs)
    res = run_bass_kernel_spmd(nc, in_maps, core_ids=list(range(N_CORES)))
    return assemble_output(
        res.results, inputs["wv_b"], inputs["wo_w"], inputs["wo_b"]
    )


# revision 49
# speedup vs baseline: 1.2008x; 1.2008x over previous
"""Trainium2 Bass kernel for 16-head MHA (B=2, T=2048, C=1024).

Sharding: 8 cores = 2 batches x 4 head-groups (4 heads each).
Each core computes, for its batch b and head group g:
  partialT[c, t] = sum_{h in g} wo[:, h].T @ (softmax(qk^T) @ v_h)^T
in fully transposed space (no on-device transposes needed):
  - host passes xT = x[b].T, weight slices pre-tiled to [128, ...] with
    4KB DMA lines, and a consts block (mask bias / Schraudolph bias /
    q,k biases) as one [128, 36] tensor
  - qT/kT computed as [d, t]; v as [t, d] (+ ones column per head for the
    softmax denominator); scores computed directly as [tk, tq]
  - denominator emerges as row 64 of the attn@v_ext matmul output;
    normalization folded in as a K=1 "replicate" matmul + multiply
  - final projection consumes the [d, t] head outputs as stationary weights
Host adds the 4 partial sums per batch, the wo bias, and the wv_b @ wo.T
constant row (v-bias contribution commutes through softmax normalization).

Schedule: heads are processed in PAIRS (SBUF partition base 0 / 64).  The
two K=64 score matmuls of a pair land on disjoint PE row-groups
(tile_position (0,0) / (64,0)) and execute concurrently, halving score
wall time.  A pass covers one 512-query chunk for both heads of a pair;
the exp runs as one [128, 1024] ACT instruction per key tile (~1.1us),
the pacing engine.  In the ACT-bound passes a few key tiles' exp runs on
the Vector engine instead, via the Schraudolph bit trick
(int32(A*(s+bias+B/A)) reinterpreted as float32, end-to-end rel err
~3e-3): one tensor_scalar into an int32 tile consumed through a
bitcast AP.  Projection / v / norm work fills the remaining PE slack per
round; softmax reciprocals run at [128, 4] after a scatter DMA.  Output
partials are stored fp16; po stores ride the gpsimd queue so the norm
DMAs on sync are never queued behind them.
"""

import sys

sys.path.insert(0, "/opt/trn_rl_repo")

import numpy as np

# ---- problem constants (hardcoded per harness contract) ----
B = 2
T = 2048
C = 1024
NUM_HEADS = 16
G = 4                 # head groups (tensor-parallel dimension)
HPG = NUM_HEADS // G  # 4 heads per core
DH = C // NUM_HEADS   # 64
DC = HPG * DH         # 256 dims per core
VE = HPG * (DH + 1)   # 260: per head 64 v-dims + 1 ones column
N_CORES = B * G       # 8
PAD_ID = 0.0

CH = 512              # tq chunk (one PSUM bank of fp32)
NCH = T // CH         # 4
NT = T // 128         # 16 token tiles
KT = C // 128         # 8 contraction tiles for projections
DM = DC // 128        # 2 m-tiles for q/k (== head pairs)

PASS_ORDER = [(0, 0), (0, 1), (1, 0), (1, 1), (0, 2), (0, 3), (1, 2), (1, 3)]

# Schraudolph exp constants (exp(x) ~ bitcast(int32(A*x + B)))
EXP_A = float((1 << 23) / np.log(2.0))
EXP_B = float((127 << 23) - 400000)
# DVE-offloaded key tiles per pass (pass idx -> rounds); heavier in the
# filler-light passes 5/6 where ACT would otherwise pace the rounds
DVE_EXP = {2: (3, 7, 11, 15), 3: (7, 13), 4: (7, 13),
           5: (3, 6, 9, 11, 13, 15), 6: (3, 6, 9, 11, 13, 15), 7: (7, 13)}


def build_nc(debug=False):
    import concourse.tile as tile
    from concourse import bacc, mybir

    f32 = mybir.dt.float32
    f32r = mybir.dt.float32r
    f16 = mybir.dt.float16
    i32 = mybir.dt.int32
    Exp = mybir.ActivationFunctionType.Exp
    add = mybir.AluOpType.add
    mult = mybir.AluOpType.mult

    nc = bacc.Bacc(
        "TRN2", target_bir_lowering=False, debug=debug, num_devices=N_CORES
    )

    xT_d = nc.dram_tensor("xT", [C, T], f16, kind="ExternalInput")
    wqT_d = nc.dram_tensor("wqT", [128, KT * DC], f16, kind="ExternalInput")
    wkT_d = nc.dram_tensor("wkT", [128, KT * DC], f16, kind="ExternalInput")
    wvT_d = nc.dram_tensor("wvT", [128, KT * VE], f16, kind="ExternalInput")
    woT_d = nc.dram_tensor("woT", [DC, C], f16, kind="ExternalInput")
    ones_d = nc.dram_tensor("ones", [128, DH], f32r, kind="ExternalInput")
    consts_d = nc.dram_tensor("consts", [128, 36], f32, kind="ExternalInput")
    outT_d = nc.dram_tensor("outT", [C, T], f16, kind="ExternalOutput")

    from contextlib import ExitStack

    with tile.TileContext(nc) as tc, ExitStack() as stack:
        persist = stack.enter_context(tc.tile_pool(name="persist", bufs=1))
        psum = stack.enter_context(tc.tile_pool(name="psum", bufs=1, space="PSUM"))
        xpool = stack.enter_context(tc.tile_pool(name="xpool", bufs=1))
        atpool = stack.enter_context(tc.tile_pool(name="atpool", bufs=1))
        work = stack.enter_context(tc.tile_pool(name="work", bufs=1))

        # PSUM bank plan (8 banks of [128, 2KB]):
        #   sc  : [128, 1024] f32 x bufs=2  -> 4 banks (score pair tiles)
        #   un0 : [128, 512] f32 x bufs=1   -> 1 bank  (attn@v head 0)
        #   un1 : [128, 512] f32 x bufs=1   -> 1 bank  (attn@v head 1)
        #   pp  : [128, 512] f32 x bufs=2   -> 2 banks (q/k/v/wo proj + rb)

        # ---------- loads (sync queue: consts + packed weights) ----------
        consts = persist.tile([128, 36], f32, name="consts", tag="consts")
        nc.sync.dma_start(consts[:, :], consts_d.ap()[:, :])
        mb = consts[:, 0:NT]                  # ACT exp bias (mask - 2)
        mbB = consts[:, NT:2 * NT]            # DVE exp bias (mask - 2 + B/A)
        bqt = [consts[:, 32 + m:33 + m] for m in range(DM)]
        bkt = [consts[:, 34 + m:35 + m] for m in range(DM)]

        # x half-tile helper (A = cols 0:1024, B = cols 1024:2048); sync is
        # the fastest queue (~190GB/s) so the ramp-gating halves ride it
        # interleaved with the weights; scalar measures only ~55GB/s
        xs = [None] * KT
        for k in range(KT):
            xs[k] = xpool.tile([128, T], f16, name=f"x{k}", tag=f"x{k}")
        HT = T // 2

        def xdma(eng, k, half):
            eng.dma_start(
                xs[k][:, half * HT:(half + 1) * HT],
                xT_d.ap()[k * 128:(k + 1) * 128, half * HT:(half + 1) * HT],
            )

        wk_t = persist.tile([128, KT * DC], f16, name="wk_t", tag="wk_t")
        nc.sync.dma_start(wk_t[:, :], wkT_d.ap()[:, :])
        for k in (0, 2, 4, 6):
            xdma(nc.scalar, k, 0)
        for k in (3, 5, 7):
            xdma(nc.gpsimd, k, 0)
        xdma(nc.sync, 1, 0)
        wq_t = persist.tile([128, KT * DC], f16, name="wq_t", tag="wq_t")
        nc.sync.dma_start(wq_t[:, :], wqT_d.ap()[:, :])
        wv_t = persist.tile([128, KT * VE], f16, name="wv_t", tag="wv_t")
        nc.sync.dma_start(wv_t[:, :], wvT_d.ap()[:, :])
        xdma(nc.sync, 1, 1)
        for k in (0, 2, 4, 6):
            xdma(nc.scalar, k, 1)
        for k in (3, 5, 7):
            xdma(nc.gpsimd, k, 1)
        ones64 = persist.tile([1, DH], f32r, name="ones64", tag="ones64")
        nc.sync.dma_start(ones64[:, :], ones_d.ap()[0:1, :])
        wo_sb = []
        for k2 in range(DM):
            wok = persist.tile([128, C], f16, name=f"wo{k2}", tag=f"wo{k2}")
            nc.sync.dma_start(wok[:, :], woT_d.ap()[k2 * 128:(k2 + 1) * 128, :])
            wo_sb.append(wok)

        # warm the ACT exp table set (~2.7us) behind the x descriptors
        expwarm = work.tile([1, 1], f32, name="expwarm", tag="expwarm", bufs=1)
        nc.vector.memset(expwarm[:, :], 0.0)
        nc.scalar.activation(expwarm[:, :], expwarm[:, :], Exp)

        # ---------- projection / v building blocks ----------
        qT = [
            persist.tile([128, T], f16, name=f"qT{m}", tag=f"qT{m}")
            for m in range(DM)
        ]
        kT = [
            persist.tile([128, T], f16, name=f"kT{m}", tag=f"kT{m}")
            for m in range(DM)
        ]
        headsT = [
            persist.tile([128, T], f16, name=f"headsT{m}", tag=f"hT{m}")
            for m in range(DM)
        ]

        def pp_tile(name):
            return psum.tile([128, CH], f32, name=name, tag="pp", bufs=2)

        # chain contraction order matched to x-tile DMA arrival
        K_ORDER = (0, 2, 4, 6, 1, 3, 5, 7)

        def proj_chain(dst, w_t, bias, m, ch):
            # one (dst, m, ch) chain: 4 two-matmul sub-units, bias evict
            state = {}
            units = []
            for step in range(4):
                def u(dst=dst, w_t=w_t, bias=bias, ch=ch,
                      state=state, m=m, step=step):
                    if step == 0:
                        state["ps"] = pp_tile(f"ps{dst[0].name}{m}{ch}")
                    ps = state["ps"]
                    for k in K_ORDER[2 * step:2 * step + 2]:
                        nc.tensor.matmul(
                            ps[:, :],
                            w_t[:, k * DC + m * 128:k * DC + (m + 1) * 128],
                            xs[k][:, ch * CH:(ch + 1) * CH],
                            start=(k == K_ORDER[0]),
                            stop=(k == K_ORDER[-1]),
                        )
                    if step == 3:
                        nc.vector.tensor_scalar_add(
                            dst[m][:, ch * CH:(ch + 1) * CH],
                            ps[:, :],
                            bias[m],
                        )
                units.append(u)
            return units

        def q_units(m, ch):
            return proj_chain(qT, wq_t, bqt, m, ch)

        def k_units(m, ch):
            return proj_chain(kT, wk_t, bkt, m, ch)

        v_sb = [None] * NT

        def v_subunits(tkt):
            state = {}
            units = []
            for step in range(2):
                def u(tkt=tkt, state=state, step=step):
                    if step == 0:
                        state["ps"] = pp_tile(f"psv{tkt}")
                    psv = state["ps"]
                    for k in K_ORDER[4 * step:4 * step + 4]:
                        nc.tensor.matmul(
                            psv[:, 0:VE],
                            xs[k][:, tkt * 128:(tkt + 1) * 128],
                            wv_t[:, k * VE:(k + 1) * VE],
                            start=(k == K_ORDER[0]),
                            stop=(k == K_ORDER[-1]),
                        )
                    if step == 1:
                        vt = persist.tile(
                            [128, VE], f16, name=f"v{tkt}", tag=f"v{tkt}"
                        )
                        nc.vector.tensor_copy(vt[:, :], psv[:, 0:VE])
                        ones_cols = vt.rearrange(
                            "p (h e) -> p h e", e=DH + 1
                        )[:, :, DH]
                        nc.vector.memset(ones_cols, 1.0)
                        v_sb[tkt] = vt
                units.append(u)
            return units

        def proj_units(mc, ch):
            # wo-projection for output rows [mc*128, ...) of chunk ch,
            # split into two ~512-cycle halves; po stores on gpsimd queue
            state = {}

            def u0():
                state["pp"] = pp_tile(f"pp{mc}{ch}")
                nc.tensor.matmul(
                    state["pp"][:, :],
                    wo_sb[0][:, mc * 128:(mc + 1) * 128],
                    headsT[0][:, ch * CH:(ch + 1) * CH],
                    start=True, stop=False,
                )

            def u1():
                pp = state["pp"]
                nc.tensor.matmul(
                    pp[:, :],
                    wo_sb[1][:, mc * 128:(mc + 1) * 128],
                    headsT[1][:, ch * CH:(ch + 1) * CH],
                    start=False, stop=True,
                )
                po = work.tile(
                    [128, CH], f16, name=f"po{mc}{ch}", tag="po", bufs=4
                )
                nc.scalar.copy(po[:, :], pp[:, :])
                nc.sync.dma_start(
                    outT_d.ap()[
                        mc * 128:(mc + 1) * 128, ch * CH:(ch + 1) * CH
                    ],
                    po[:, :],
                )
            return u0, u1

        # ---------- norm (softmax denominator) ----------
        pending_norm = []
        tail_rb = {"on": False, "n": 0}

        def make_norm_part2(p, ch, hh, unev, rr):
            base = hh * 64

            def emit():
                if tail_rb["on"]:
                    # tail: pp bufs are held by proj-c3 first halves; the
                    # un banks are free after the final evicts
                    rb = psum.tile(
                        [128, CH], f32, name=f"rbt{p}{ch}{hh}",
                        tag=f"un{tail_rb['n'] % 2}", bufs=1,
                    )
                    tail_rb["n"] += 1
                else:
                    rb = pp_tile(f"rb{p}{ch}{hh}")
                nc.tensor.matmul(
                    rb[0:DH, :], ones64[:, :], rr[:, :], start=True, stop=True
                )
                if base == 0:
                    nc.vector.tensor_mul(
                        headsT[p][0:DH, ch * CH:(ch + 1) * CH],
                        unev[0:DH, :],
                        rb[0:DH, :],
                    )
                else:
                    scr = work.tile(
                        [DH, CH], f16, name=f"scr{p}{ch}{hh}", tag="scr",
                        bufs=4,
                    )
                    nc.vector.tensor_mul(scr[:, :], unev[0:DH, :], rb[0:DH, :])
                    sq = nc.sync if tail_rb["on"] else nc.gpsimd
                    sq.dma_start(
                        headsT[p][base:base + 64, ch * CH:(ch + 1) * CH],
                        scr[:, :],
                    )
            return emit

        def norm_part1(p, ch, hh, un):
            # evict un, spread denominator row across 128 partitions,
            # reciprocal there, gather back to [1, 512] for the rb matmul
            unev = work.tile(
                [DH + 1, CH], f32, name=f"unev{p}{ch}{hh}", tag="unev", bufs=6
            )
            nc.vector.tensor_copy(unev[:, :], un[0:DH + 1, :])
            dq = nc.sync if tail_rb["on"] else nc.gpsimd
            drp = work.tile(
                [128, CH // 128], f32, name=f"drp{p}{ch}{hh}", tag="drp",
                bufs=6,
            )
            dq.dma_start(drp[:, :], unev[DH:DH + 1, :])
            rrp = work.tile(
                [128, CH // 128], f32r, name=f"rrp{p}{ch}{hh}", tag="rrp",
                bufs=6,
            )
            with nc.allow_low_precision(reason="fp32r matmul operand"):
                nc.vector.reciprocal(rrp[:, :], drp[:, :])
            rr = work.tile(
                [1, CH], f32r, name=f"rr{p}{ch}{hh}", tag="rr", bufs=6
            )
            dq.dma_start(rr[:, :], rrp[:, :])
            pending_norm.append(make_norm_part2(p, ch, hh, unev, rr))

        # ---------- minimal ramp: just enough for sc(0) of pass 0 ----------
        for u in k_units(0, 0):
            u()
        for u in q_units(0, 0):
            u()

        # ---------- filler schedule: pass idx -> {round: [closures]} ----------
        filler = {pi: {} for pi in range(len(PASS_ORDER))}

        def sched(pi, t, *fns):
            filler[pi].setdefault(t, []).extend(fns)

        vu = {t: v_subunits(t) for t in range(NT)}
        # pass 0: v(0..15) [deadline: round t], kT[0] c1/c2/c3
        # [deadline: round 4c-2], qT[0] c1 [deadline: end of pass]
        k01, k02, k03 = k_units(0, 1), k_units(0, 2), k_units(0, 3)
        sched(0, 0, *vu[0], *vu[1])
        sched(0, 1, *vu[2], *k01[0:2])
        sched(0, 2, *vu[3], *k01[2:4])
        sched(0, 3, *vu[4])
        sched(0, 4, *vu[5], *k02[0:2])
        sched(0, 5, *vu[6], *k02[2:4])
        sched(0, 6, *vu[7])
        sched(0, 7, *vu[8])
        sched(0, 8, *vu[9], *k03[0:2])
        sched(0, 9, *vu[10], *k03[2:4])
        sched(0, 10, *vu[11])
        sched(0, 11, *vu[12])
        q01 = q_units(0, 1)
        sched(0, 12, *vu[13], q01[0])
        sched(0, 13, *vu[14], q01[1])
        sched(0, 14, *vu[15], q01[2])
        sched(0, 15, q01[3])

        # pass 1: kT[1] c0/c1 (1 unit/round) + qT[1] c0
        for c in range(2):
            ku = k_units(1, c)
            for j in range(4):
                sched(1, 4 * c + j, ku[j])
        q10 = q_units(1, 0)
        for j in range(4):
            sched(1, 8 + 2 * j, q10[j])
        # pass 2: kT[1] c2/c3 (deadline: chunk c read at sc(4c), emitted
        # round 4c-1), then qT[1] c1, qT[0] c2
        k12, k13 = k_units(1, 2), k_units(1, 3)
        for j in range(4):
            sched(2, j, k12[j])
            sched(2, 4 + j, k13[j])
        for j, u in enumerate(q_units(1, 1)):
            sched(2, 8 + j, u)
        # passes 3/4/7: wo-projection for chunks 0/1/2 (halves pipelined)
        for pi, ch in ((3, 0), (4, 1), (7, 2)):
            prev = None
            for mc in range(8):
                u0, u1 = proj_units(mc, ch)
                if prev is None:
                    sched(pi, 4, u0)
                else:
                    sched(pi, 4 + mc, prev, u0)
                prev = u1
            sched(pi, 12, prev)
        for j, u in enumerate(q_units(0, 2)):
            sched(3, 12 + j, u)
        for j, u in enumerate(q_units(0, 3)):
            sched(4, 12 + j, u)
        for j, u in enumerate(q_units(1, 3)):
            sched(5, 2 + 2 * j, u)
        for j, u in enumerate(q_units(1, 2)):
            sched(5, 10 + j, u)

        # norm part2 pops: pass i's norms pop early in pass i+1 (pass 0's
        # and 1's in pass 2).  In proj passes (3/4/7) both pops must land
        # before round 4, where the first proj unit reads headsT.
        pops = {2: (1, 5, 9, 13), 3: (1, 3), 4: (1, 3), 5: (1, 5),
                6: (1, 5), 7: (1, 3)}

        # ---------- attention passes ----------
        tail_units = [proj_units(mc, 3) for mc in range(8)]
        for pi, (p, ch) in enumerate(PASS_ORDER):
            un = [
                psum.tile([128, CH], f32, name=f"un{p}{ch}{hh}",
                          tag=f"un{hh}", bufs=1)
                for hh in range(2)
            ]
            dve_rounds = DVE_EXP.get(pi, ())

            def emit_sc(t, p=p, ch=ch):
                sc = psum.tile(
                    [128, 2 * CH], f32, name=f"sc{p}{ch}t{t}", tag="sc",
                    bufs=2,
                )
                for hh in range(2):
                    base = hh * 64
                    nc.tensor.matmul(
                        sc[:, hh * CH:(hh + 1) * CH],
                        kT[p][base:base + 64, t * 128:(t + 1) * 128],
                        qT[p][base:base + 64, ch * CH:(ch + 1) * CH],
                        start=True,
                        stop=True,
                    )
                return sc

            sc_cur = emit_sc(0)
            for t in range(NT):
                if t in dve_rounds:
                    # Schraudolph exp on DVE: int32((sc + mbB)*A), read back
                    # through a f32 bitcast and narrowed to f16 for the
                    # attn@v matmul (PE rejects f16-stationary/f32-moving)
                    ati = atpool.tile(
                        [128, 2 * CH], i32, name=f"ati{p}{ch}t{t}", tag="ati",
                        bufs=2,
                    )
                    nc.vector.tensor_scalar(
                        out=ati[:, :], in0=sc_cur[:, :],
                        scalar1=mbB[:, t:t + 1], scalar2=EXP_A,
                        op0=add, op1=mult,
                    )
                    at = atpool.tile(
                        [128, 2 * CH], f16, name=f"at{p}{ch}t{t}", tag="at",
                        bufs=6,
                    )
                    nc.vector.tensor_copy(at[:, :], ati[:, :].bitcast(f32))
                    at_slices = [at[:, hh * CH:(hh + 1) * CH] for hh in range(2)]
                else:
                    at = atpool.tile(
                        [128, 2 * CH], f16, name=f"at{p}{ch}t{t}", tag="at",
                        bufs=6,
                    )
                    nc.scalar.activation(
                        at[:, :], sc_cur[:, :], Exp, bias=mb[:, t:t + 1]
                    )
                    at_slices = [at[:, hh * CH:(hh + 1) * CH] for hh in range(2)]
                if t + 1 < NT:
                    sc_cur = emit_sc(t + 1)
                for fn in filler[pi].get(t, ()):
                    fn()
                if t in pops.get(pi, ()) and pending_norm:
                    pending_norm.pop(0)()
                for hh in range(2):
                    h = 2 * p + hh
                    nc.tensor.matmul(
                        un[hh][0:DH + 1, :],
                        v_sb[t][:, h * (DH + 1):(h + 1) * (DH + 1)],
                        at_slices[hh],
                        start=(t == 0),
                        stop=(t == NT - 1),
                    )
                    if t == NT - 1 and pi < 7:
                        norm_part1(p, ch, hh, un[hh])

            if pi == 7:
                # last pass: proj-c3 first halves go out BEFORE the norm
                # DMAs (queue-counting semaphores would otherwise gate the
                # matmuls on them), then the final norms on the idle sync
                # queue
                tail_units[0][0]()
                tail_units[1][0]()
                tail_rb["on"] = True
                for hh in range(2):
                    norm_part1(p, ch, hh, un[hh])

        # ---------- tail: wo-proj chunk 3 ----------
        while pending_norm:
            pending_norm.pop(0)()
        for mc in range(8):
            tail_units[mc][1]()
            if mc + 2 < 8:
                tail_units[mc + 2][0]()

    nc.compile()
    return nc


def make_in_maps(x, wq_w, wq_b, wk_w, wk_b, wv_w, wv_b, wo_w, wo_b):
    scale = DH ** -0.5

    def pack(wT, ve):  # [C, ve*KT-ish] -> [128, KT*ve] tiled rows
        return np.ascontiguousarray(
            wT.reshape(KT, 128, ve).transpose(1, 0, 2).reshape(128, KT * ve)
        )

    in_maps = []
    for c in range(N_CORES):
        b, g = divmod(c, G)
        sl = slice(g * DC, (g + 1) * DC)
        wvT_ext = np.zeros((C, VE), np.float32)
        for hl in range(HPG):
            rows = slice(g * DC + hl * DH, g * DC + (hl + 1) * DH)
            wvT_ext[:, hl * (DH + 1):hl * (DH + 1) + DH] = wv_w[rows, :].T
        # consts [128, 36]: mask bias, Schraudolph bias, bq, bk
        col0 = np.asarray(x[b][:, 0], np.float32).reshape(NT, 128).T
        mbias = np.where(col0 == PAD_ID, -1e30, 0.0).astype(np.float32) - 2.0
        consts = np.zeros((128, 36), np.float32)
        consts[:, 0:NT] = mbias
        consts[:, NT:2 * NT] = mbias + np.float32(EXP_B / EXP_A)
        for m in range(DM):
            consts[:, 32 + m] = (wq_b[sl] * scale)[m * 128:(m + 1) * 128]
            consts[:, 34 + m] = wk_b[sl][m * 128:(m + 1) * 128]
        in_maps.append({
            "xT": np.ascontiguousarray(x[b].T).astype(np.float16),
            "wqT": pack((wq_w[sl] * scale).T, DC).astype(np.float16),
            "wkT": pack(wk_w[sl].T, DC).astype(np.float16),
            "wvT": pack(wvT_ext, VE).astype(np.float16),
            "woT": np.ascontiguousarray(wo_w[:, sl].T).astype(np.float16),
            "ones": np.ones((128, DH), np.float32),
            "consts": consts,
        })
    return in_maps


def assemble_output(results, wv_b, wo_w, wo_b):
    const_row = wv_b @ wo_w.T + wo_b  # [C]
    out = np.zeros((B, T, C), np.float32)
    for c in range(N_CORES):
        b = c // G
        out[b] += results[c]["outT"].astype(np.float32).T
    out += const_row[None, None, :]
    return out.astype(np.float32)


_nc_cache = {}


def kernel(**inputs):
    from concourse.bass_utils import run_bass_kernel_spmd

    if "nc" not in _nc_cache:
        _nc_cache["nc"] = build_nc(debug=False)
    nc = _nc_cache["nc"]

    in_maps = make_in_maps(**inputs)
    res = run_bass_kernel_spmd(nc, in_maps, core_ids=list(range(N_CORES)))
    return assemble_output(
        res.results, inputs["wv_b"], inputs["wo_w"], inputs["wo_b"]
    )
